# revision 39
# baseline (speedup 1.0000x reference)
"""Trainium2 Bass kernel for nn_BilinearInterpolator (dense per-coord CNN).

Math (per (b, n) pair):
  u      = w1[:, :5] @ [image_b; pos]              # [64, 1024], shared over n
  v      = w1[:, 5:] @ coords[b, n] + b1           # [64] per-pair bias
  h1     = leaky(u + v)                            # [64, 1024]
  h_l    = leaky(W_l h_{l-1} + b_l)   l = 2..5
  pooled = mean_hw(h5);  out = sigmoid(wl @ pooled + bl)

Sharding: 512 (b, n) pairs data-parallel over 8 cores (64 pairs each; every
core owns a single b). On-chip layout packs 2 pairs per 128-partition tile
(channels 0-63 = even pair, 64-127 = odd pair); all matmuls use block-diagonal
[128, 128] weights.

The tiny shared tensors u (one [64,1024] map per core) and v (64 scalars
per pack) are precomputed on host, as is the final head: the device only
runs the per-pack pipeline whose cost actually scales with B*N*HW.

Engine split (the per-layer PSUM drains are the bottleneck; ScalarE and
VectorE must share them):
  L1   -> VectorE (u is fp16 SBUF: add 4x, mask 4x, mult 2x)
  L2-4 -> ScalarE fused Prelu; L4 additionally emits accum_out -> pooled4.
  L5   -> VectorE, ONE op: min(z5, -b5) cache-reduce accum -> pneg.
          Using leaky(a) = a - 0.9*min(a, 0) and sum(z5) = W5 @ pooled4,
          the pooled head is reassembled on host from pooled4 and pneg -
          no h5/a5 materialization at all.
  A few L2 tiles run on VectorE (3-op leaky) to balance the engines.
Stages are emitted pair-granular in a skewed wavefront (only even t for
l >= 2, odd SKEW) so the 8-bank PSUM ring holds exactly one wave of z tiles
and every buffer is freed in the wave that allocates it.
"""

import sys

if "/opt/trn_rl_repo" not in sys.path:
    sys.path.insert(0, "/opt/trn_rl_repo")

import numpy as np

import concourse.mybir as mybir
from concourse.bacc import Bacc
from concourse import tile
from concourse.bass_utils import run_bass_kernel_spmd

B, N, H, W, C = 4, 128, 32, 32, 64
HW = H * W
NCORES = 8
PAIRS = (B * N) // NCORES  # 64 pairs per core
PACKS = PAIRS // 2  # 32 packed tiles per core
NEG = 0.1
F32 = mybir.dt.float32
F16 = mybir.dt.float16
MM_DT = F16

A = mybir.ActivationFunctionType
OP = mybir.AluOpType

SKEW = 3


def _dve23(l, tt):
    # L2 tiles drained on VectorE for load balance; L2 stages land on odd
    # waves where VectorE is otherwise idle.
    return l == 2 and tt % 3 == 2


def _build():
    nc = Bacc()
    d = {}
    for name, shape, dt in [
        ("udup", [128, HW], MM_DT),
        ("bias1", [128, PACKS], F32),
        ("bball", [128, 4], F32),
        ("bb5n", [128, 1], F32),
        ("wall", [128, 4 * 128], MM_DT),
    ]:
        d[name] = nc.dram_tensor(name, shape, dt, kind="ExternalInput")
    p4_d = nc.dram_tensor("pooled4", [128, PACKS], F32, kind="ExternalOutput")
    pn_d = nc.dram_tensor("pneg", [128, PACKS], F32, kind="ExternalOutput")

    with tile.TileContext(nc) as tc:
        with (
            tc.tile_pool(name="consts", bufs=1) as consts,
            tc.tile_pool(name="hpool", bufs=14) as hpool,
            tc.tile_pool(name="apool", bufs=5) as apool,
            tc.tile_pool(name="mpool", bufs=6) as mpool,
            tc.tile_pool(name="zpool", bufs=4, space="PSUM") as zpool,
        ):
            # Warm the Prelu spline table while input DMAs are in flight.
            warm = consts.tile([128, 1], F32, tag="warm")
            nc.vector.memset(warm[:], 0.0)
            nc.scalar.activation(warm[:], warm[:], A.Prelu, scale=1.0, alpha=NEG)

            sb = {}
            for name in d:
                sb[name] = consts.tile(list(d[name].shape), d[name].dtype, tag=name, name="sb_" + name)
                nc.sync.dma_start(sb[name][:], d[name][:])

            w_l = {l: sb["wall"][:, 128 * (l - 2) : 128 * (l - 1)] for l in (2, 3, 4, 5)}
            bb_l = {l: sb["bball"][:, (l - 2) : (l - 1)] for l in (2, 3, 4, 5)}
            u_dup = sb["udup"]
            bias1 = sb["bias1"]

            pooled4 = consts.tile([128, PACKS], F32, tag="pooled4")
            pneg = consts.tile([128, PACKS], F32, tag="pneg")

            hcur = {}

            def stage1(t):
                # First two pairs run on ScalarE (fused Prelu) — it is
                # otherwise idle during pipeline fill; rest on VectorE.
                if t < 4:
                    h = hpool.tile([128, 2 * HW], MM_DT, tag="h", name=f"h1_{t}")
                    for i, tt in enumerate((t, t + 1)):
                        nc.scalar.activation(
                            h[:, i * HW : (i + 1) * HW], u_dup[:], A.Prelu,
                            bias=bias1[:, tt : tt + 1], scale=1.0, alpha=NEG,
                        )
                    hcur[t] = h[:, 0:HW]
                    hcur[t + 1] = h[:, HW : 2 * HW]
                    return
                # packs t, t+1 on VectorE: two per-pack bias adds into one
                # [128, 2*HW] tile, then a single paired mask and mult.
                a = apool.tile([128, 2 * HW], MM_DT, tag="a", name=f"a1_{t}")
                for i, tt in enumerate((t, t + 1)):
                    nc.vector.tensor_scalar(
                        a[:, i * HW : (i + 1) * HW], u_dup[:],
                        bias1[:, tt : tt + 1], None, OP.add,
                    )
                m = mpool.tile([128, 2 * HW], MM_DT, tag="m", name=f"m1_{t}")
                nc.vector.tensor_scalar(m[:], a[:], 0.0, NEG, OP.is_ge, OP.max)
                h = hpool.tile([128, 2 * HW], MM_DT, tag="h", name=f"h1_{t}")
                nc.vector.tensor_tensor(h[:], a[:], m[:], OP.mult)
                hcur[t] = h[:, 0:HW]
                hcur[t + 1] = h[:, HW : 2 * HW]

            def stage(l, t):
                # layers 2..5 for packs t, t+1
                zs = {}
                for tt in (t, t + 1):
                    h = hcur.pop(tt)
                    z = zpool.tile([128, HW], F32, tag="z", name=f"z{l}_{tt}")
                    for c0 in (0, 512):
                        nc.tensor.matmul(
                            z[:, c0 : c0 + 512], w_l[l], h[:, c0 : c0 + 512],
                            start=True, stop=True, skip_group_check=True,
                        )
                    zs[tt] = z
                if l == 5:
                    for tt in (t, t + 1):
                        scr = mpool.tile([128, HW], MM_DT, tag="m", name=f"r5_{tt}")
                        nc.vector.tensor_scalar(
                            scr[:], zs[tt][:], sb["bb5n"][:], 0.0, OP.min, OP.add,
                            accum_out=pneg[:, tt : tt + 1],
                        )
                    return
                for tt in (t, t + 1):
                    z = zs[tt]
                    if _dve23(l, tt):
                        a = apool.tile([128, HW], MM_DT, tag="a", name=f"a{l}_{tt}")
                        nc.vector.tensor_scalar(a[:], z[:], bb_l[l], None, OP.add)
                        m = mpool.tile([128, HW], MM_DT, tag="m", name=f"m{l}_{tt}")
                        nc.vector.tensor_scalar(m[:], a[:], 0.0, NEG, OP.is_ge, OP.max)
                        hn = hpool.tile([128, HW], MM_DT, tag="h", name=f"h{l}_{tt}")
                        nc.vector.tensor_tensor(hn[:], a[:], m[:], OP.mult)
                    else:
                        hn = hpool.tile([128, HW], MM_DT, tag="h", name=f"h{l}_{tt}")
                        if l == 4:
                            nc.scalar.activation(
                                hn[:], z[:], A.Prelu,
                                bias=bb_l[l], scale=1.0, alpha=NEG,
                                accum_out=pooled4[:, tt : tt + 1],
                            )
                        else:
                            nc.scalar.activation(
                                hn[:], z[:], A.Prelu,
                                bias=bb_l[l], scale=1.0, alpha=NEG,
                            )
                    hcur[tt] = hn

            # l=5 emitted before l=3 so PE runs the mm5s first within each
            # even wave: the r5 drains sit early in VectorE's wave queue and
            # would otherwise stall on matmuls scheduled at the wave's end.
            for w in range(PACKS + SKEW * 4 + 1):
                for l in (1, 5, 2, 3, 4):
                    t = w - SKEW * (l - 1)
                    if 0 <= t < PACKS and t % 2 == 0:
                        if l == 1:
                            stage1(t)
                        else:
                            stage(l, t)

            # Copies on the producing engines: their FIFO order guarantees all
            # accumulator-read aux ops have retired before the DMA source is
            # materialized (hardens against aux-op/DMA ordering races).
            p4c = consts.tile([128, PACKS], F32, tag="p4c")
            nc.scalar.copy(p4c[:], pooled4[:])
            pnc = consts.tile([128, PACKS], F32, tag="pnc")
            nc.vector.tensor_scalar(pnc[:], pneg[:], 1.0, None, OP.mult)
            nc.sync.dma_start(p4_d[:], p4c[:])
            nc.sync.dma_start(pn_d[:], pnc[:])

    nc.compile()
    return nc


_CACHE = {}


def _get_nc():
    if "nc" not in _CACHE:
        _CACHE["nc"] = _build()
    return _CACHE["nc"]


def _prep_core_inputs(image, coords, w1, b1, ws, bs, core):
    b = core // 2
    n0 = (core % 2) * PAIRS

    row = (np.arange(H, dtype=np.float32) / (H - 1))[:, None] * np.ones(
        (1, W), np.float32
    )
    col = np.ones((H, 1), np.float32) * (np.arange(W, dtype=np.float32) / (W - 1))[None]
    pos = np.stack([row, col], 0).reshape(2, HW)
    xin = np.concatenate([image[b].reshape(3, HW), pos], 0)  # [5, HW]

    u = w1[:, :5] @ xin  # [64, HW]
    udup = np.concatenate([u, u], 0).astype(np.float16)  # [128, HW]

    cs = coords[b, n0 : n0 + PAIRS]  # [64, 2]
    v = cs @ w1[:, 5:].T + b1  # [64 pairs, 64 ch]
    bias1 = np.empty((128, PACKS), np.float32)
    bias1[0:64] = v[0::2].T
    bias1[64:128] = v[1::2].T

    wall = np.zeros((128, 4 * 128), np.float32)
    bball = np.zeros((128, 4), np.float32)
    for i, (w, bias) in enumerate(zip(ws, bs)):
        wall[0:64, 128 * i : 128 * i + 64] = w.T
        wall[64:128, 128 * i + 64 : 128 * i + 128] = w.T
        bball[:, i] = np.concatenate([bias, bias])

    b5 = bs[3]
    return {
        "udup": udup,
        "bias1": bias1,
        "wall": wall.astype(np.float16),
        "bball": bball,
        "bb5n": np.concatenate([-b5, -b5]).reshape(128, 1).astype(np.float32),
    }


def _run(inputs, trace=False):
    image = np.asarray(inputs["image"], np.float32)
    coords = np.asarray(inputs["coords"], np.float32)
    w1 = np.asarray(inputs["w1"], np.float32)
    b1 = np.asarray(inputs["b1"], np.float32)
    ws = [np.asarray(inputs[f"w{i}"], np.float32) for i in (2, 3, 4, 5)]
    bs = [np.asarray(inputs[f"b{i}"], np.float32) for i in (2, 3, 4, 5)]
    wl = np.asarray(inputs["wl"], np.float32)
    bl = np.asarray(inputs["bl"], np.float32)

    nc = _get_nc()
    in_maps = [
        _prep_core_inputs(image, coords, w1, b1, ws, bs, c) for c in range(NCORES)
    ]
    res = run_bass_kernel_spmd(nc, in_maps, list(range(NCORES)), trace=trace)

    # Host head: sum_pos leaky(a5) = W5 @ pooled4 - 0.9*pneg_raw + 0.1*HW*b5
    w5, b5 = ws[3], bs[3]
    pred = np.empty((B, 3, N), np.float32)
    for c in range(NCORES):
        b = c // 2
        n0 = (c % 2) * PAIRS
        p4 = res.results[c]["pooled4"]  # [128, PACKS]
        pn = res.results[c]["pneg"]  # [128, PACKS]
        for half, off in ((0, 0), (1, 1)):
            s = slice(64 * half, 64 * half + 64)
            sl = w5 @ p4[s] - (1 - NEG) * pn[s] + NEG * HW * b5[:, None]
            logits = wl @ (sl / HW) + bl[:, None]  # [3, PACKS]
            pred[b, :, n0 + off : n0 + PAIRS : 2] = 1 / (1 + np.exp(-logits))
    return pred, res


def kernel(**inputs) -> np.ndarray:
    pred, _ = _run(inputs, trace=False)
    return pred
